# revision 43
# baseline (speedup 1.0000x reference)
"""Trainium2 Bass kernel for nn_AttentionalGNN (gnn_message_passing).

Algebraic collapse (exact): in the reference, src[e] = x[row[row[e]]] and
dst[e] = x[row[col[e]]], so the 4000x4000 edge attention collapses to a
200x200 node attention with multiplicative key weights cb[w] (applied as a
ln(cb) bias on the exp) and the scatter-add collapses to a 200x200 count
matrix M. lin/q/k/v/o fold on the host into G (Gram: scores are x^T G x),
RV (query-side bias), VO (o-projected values).

Fast path (_build_program_1c, 14135ns vs the 16328ns prior baseline):
liveQ = unique(row[row]) and liveK = unique(row[col]) are both subsets of
unique(row[:200]) (~126 of 200), so queries and keys share ONE <=128
union-compacted x: one x pack, one 128-wide L1 scatter + relu, and all
attention loops are single-chunk.

Timeline-model-driven layout (every stage verified at the TRN2 cost-model
floor -- DMA first-use pays desc-gen 625 + DGE delay 650 + sem-prop 900ns,
each cross-engine hop ~105-240ns):
* pack1 (x|G|rv4) is fp8-e4m3 (quantization adds ~1e-3 rel err, 14x under
  the gate; halves the first transfer). fp8 stationary x bf16 moving
  matmuls (scores, vo) are legal on HW. ln(count) ships precomputed as a
  bf16 bias column with VO1; counts stay exact.
* the rv bias never touches the th stage: scores = x^T(G^T x) + B^T 1
  with B = rv^T x (one 4-col matmul); B enters the scores PSUM as a
  rank-4 opener against a constant head-block selector E4, hidden in the
  PE idle window. E4 + all rank-1 rows ride one SWDGE transfer (Pool
  queue) in parallel with the serial HWDGE input stream.
* PSUM banks are all distinct (th-hp0 / th-hp1+apt-rotation / vo / sc
  (B,sp) / apt / sps1 / sps2 / warm-up), so the two th copies run
  genuinely parallel on DVE+ACT and nothing serializes on bank reuse.
* both scatter openers (bc deg rank-1s) hoist right after th; scatter
  accumulates into long-open groups closed by the 4 message matmuls.
* the kv_writeback descriptor prep (~1us SWDGE) runs at kernel start
  under the DMA shadow. Post-compile patches (the scheduler otherwise
  floats the dep-free trigger next to the prep, which would DMA stale
  zeros): the trigger gets the final sub's DVE-counter wait, the Pool
  gate copy is deleted (downstream Pool-counter waits decremented), and
  the ysem wait parks late in the END block so the 900ns DMA-completion
  propagation overlaps the epilogue drains. The framework's stale DMASW
  epilogue waits are stripped; ysem provides the ordering guarantee.
* ~11 warm-up matmuls on a spare bank bridge the PE p-state ramp on real
  HW (free in the cost model: the ramp anchors at PE queue start).
"""

import numpy as np

N = 200          # nodes
C = 128          # channels
H = 4            # heads
DH = 128         # head dim
QKV = 512        # H * DH
_CHUNKS = ((0, 128), (128, 72))   # multi-chunk fallback path
N_CORES = 8
QP = 128         # padded compact union axis (fast path)

PACKA = N + QKV             # mc: xT | G1
PACKB = QKV + 2 * N         # mc: VO1 | MT0 | MT1
PACKC = QKV + QKV           # mc: G2 | VO2
PACKROW = QKV + QKV + N + C + C   # mc: rv1 | rv2 | deg | bc1 | bc2

_CACHE = {}


def _patch_writeback_order(nc):
    """The scheduler orders the Pool queue as prep -> trigger -> wait(ysem)
    (the trigger has no data deps, so it floats right after the prep and
    would fire the output DMA with stale data). Give the trigger an explicit
    wait on the DVE engine counter at the final out_sb sub's completion
    value, drop the redundant Pool gate copy, and move trigger+wait to the
    end of the block."""
    import concourse.mybir as mybir

    for blk in nc.m.functions[0].blocks:
        insts = blk.instructions
        t_idx = next((i for i, x in enumerate(insts)
                      if type(x).__name__ == "InstTriggerDma"), None)
        if t_idx is None:
            continue
        trig = insts[t_idx]
        w_idx = next(i for i, x in enumerate(insts)
                     if isinstance(x, mybir.InstEventSemaphore)
                     and x.sync_info is not None
                     and any(w.ant_name == "ysem"
                             for w in (x.sync_info.on_wait or [])))
        wait = insts[w_idx]
        g_idx = next(i for i, x in enumerate(insts)
                     if isinstance(x, mybir.InstTensorCopy)
                     and getattr(x, "engine", None) == mybir.EngineType.Pool)
        gate = insts[g_idx]
        assert t_idx < w_idx < g_idx, (t_idx, w_idx, g_idx)
        # the gate copy's DVE wait IS the sub-completion condition; move it
        # onto the trigger itself and drop the gate
        dve_waits = [w for w in (gate.sync_info.on_wait or [])
                     if w.ant_name.startswith("DVE")]
        assert dve_waits, "gate copy lost its DVE wait"
        # ISA ops take a single sync wait: replace the Pool-counter wait
        # (prep ordering is already guaranteed by the queue: the prep
        # finishes ~10us before the sub lands) with the sub's DVE wait
        trig.sync_info = mybir.SyncInfo(
            on_wait=dve_waits,
            on_update=list(trig.sync_info.on_update or []))
        # the gate was a Pool_49 incrementer; find its ordinal among the
        # increments and decrement any wait thresholds that counted it
        ordinal = 0
        for b2 in nc.m.functions[0].blocks:
            done = False
            for x in b2.instructions:
                if x.sync_info is not None:
                    for u in (x.sync_info.on_update or []):
                        if u.ant_name == "Pool_49":
                            ordinal += u.update_value
                if x is gate:
                    done = True
                    break
            if done:
                break
        for b2 in nc.m.functions[0].blocks:
            for x in b2.instructions:
                if x is gate or x.sync_info is None:
                    continue
                for w in (x.sync_info.on_wait or []):
                    if w.ant_name == "Pool_49" and w.wait_value >= ordinal:
                        w.wait_value -= 1
        del insts[g_idx]
        del insts[w_idx]
        del insts[t_idx]
        # keep the block terminator (branch) last
        end = len(insts)
        while end > 0 and type(insts[end - 1]).__name__ in (
                "InstUnconditionalBranch", "InstEventSemaphore", "InstDrain"):
            end -= 1
        insts.insert(end, trig)
        # Park the ysem wait late in the END block (before the last Pool
        # barrier) so the ~900ns DMA-completion sem propagation overlaps
        # the epilogue drains instead of preceding them.
        blocks = nc.m.functions[0].blocks
        endblk = blocks[-1]
        pool_idxs = [i for i, x in enumerate(endblk.instructions)
                     if getattr(x, "engine", None) == mybir.EngineType.Pool]
        last_two_barriers = [i for i in pool_idxs
                             if isinstance(endblk.instructions[i],
                                           mybir.InstEventSemaphore)][-2:]
        endblk.instructions.insert(last_two_barriers[0], wait)
        return
    raise AssertionError("trigger/gate/wait pattern not found")


def _patch_dmasw(nc):
    """Drop the framework's stale DMASW epilogue waits (the kv_writeback
    prep is tracked on the DMASW0 lane but completes on ysem; the explicit
    wait_ge(ysem) provides the ordering guarantee)."""
    import concourse.mybir as mybir

    for blk in nc.m.functions[0].blocks:
        for inst in blk.instructions:
            si = inst.sync_info
            if si is None or not isinstance(inst, mybir.InstEventSemaphore):
                continue
            waits = list(si.on_wait or [])
            keep = [w for w in waits
                    if not (w.ant_name or "").startswith("DMASW")]
            if len(keep) != len(waits):
                inst.sync_info = mybir.SyncInfo(
                    on_wait=keep, on_update=list(si.on_update or []))


def _build_program_1c():
    """Single-chunk union-compacted program (see module docstring)."""
    import concourse.mybir as mybir
    import concourse.tile as tile
    from concourse import bacc

    f32 = mybir.dt.float32
    bf16 = mybir.dt.bfloat16
    f8 = mybir.dt.float8e4
    AF = mybir.ActivationFunctionType

    P1 = QP + QKV + 8        # xU | G1 | RV4(L1) | RV4(L2)
    P2A = QKV + 1            # VO1 | ln(cnt)
    P2B = QP + N             # MTu | MTf
    P3 = 2 * QKV             # G2 | VO2
    PROW = QP + N + 2 * C    # degU | degF | bc1 | bc2 (row 0 of packE4row)
    PE4R = 4 * QP + PROW     # E4 | packRow-in-row-0

    nc = bacc.Bacc("TRN2", target_bir_lowering=False)

    din = {}
    for name, shape, dt_ in (
        ("pack1", [C, P1], f8),
        ("pack2a", [C, P2A], bf16),
        ("packE4", [4, PE4R], bf16),
        ("pack2b", [C, P2B], bf16),
        ("pack3", [C, P3], bf16),
    ):
        din[name] = nc.dram_tensor(name, shape, dt_, kind="ExternalInput")
    y_d = nc.dram_tensor("yT", [1, C, 1, N], f32, kind="ExternalOutput")

    with tile.TileContext(nc) as tc:
        with (
            tc.tile_pool(name="singles", bufs=1) as singles,
            tc.tile_pool(name="work", bufs=2) as work,
            tc.tile_pool(name="psum", bufs=1, space="PSUM") as psum,
        ):
            # --- Pool queue: packRow via SWDGE (parallel to HWDGE), then
            # the writeback descriptor prep under the DMA shadow ---
            E4t = singles.tile([4, PE4R], bf16, tag="w_E4")
            nc.gpsimd.dma_start(E4t[:], din["packE4"][:])
            prow = E4t[0:1, 4 * QP:]
            zidx = singles.tile([128, 1], mybir.dt.int32, tag="w_zidx")
            nc.gpsimd.memset(zidx[:], 0)
            ones_bf = singles.tile([1, N], bf16, tag="w_ones_bf")
            nc.vector.memset(ones_bf[:], 1.0)
            out_sb = singles.tile([128, N], f32, tag="w_out_sb")
            nc.vector.memset(out_sb[:], 0.0)
            ysem = nc.alloc_semaphore("ysem")
            nc.gpsimd.kv_writeback(
                y_d[:], out_sb[:].rearrange("p (a b n) -> p a b n", a=1, b=1),
                zidx[:], prepare_only=True, sem=ysem)

            # --- PE p-state ramp on a dedicated bank: one tile, closed
            # back-to-back groups (no pool rotation => no WAW semaphores),
            # keeps the PE busy from ~1us until the first weight DMA
            # lands (~3.35us) so the 3us clock ramp happens under the
            # DMA shadow ---
            jk = psum.tile([128, 512], f32, tag="apx", bufs=1)
            for _ in range(11):
                nc.tensor.matmul(jk[:, :N], ones_bf[:1, :128],
                                 ones_bf[:1, :N], start=True, stop=True)

            # ACT table: func-set 6 holds exp, ln AND copy.
            nc.scalar.add_instruction(mybir.InstLoadActFuncSet(
                act_func_set_id=6,
                name=nc.get_next_instruction_name(),
                ins=[], outs=[]))

            # --- HWDGE input DMAs, in need order (desc-gen serializes) ---
            p1 = singles.tile([C, P1], f8, tag="w_p1")
            nc.sync.dma_start(p1[:], din["pack1"][:])
            p2a = singles.tile([C, P2A], bf16, tag="w_p2a")
            nc.sync.dma_start(p2a[:], din["pack2a"][:])
            p2b = singles.tile([C, P2B], bf16, tag="w_p2b")
            nc.sync.dma_start(p2b[:], din["pack2b"][:])
            p3 = singles.tile([C, P3], bf16, tag="w_p3")
            nc.sync.dma_start(p3[:], din["pack3"][:])

            xU = p1[:, 0:QP]
            W = {"G1": p1[:, QP:QP + QKV], "VO1": p2a[:, 0:QKV],
                 "G2": p3[:, 0:QKV], "VO2": p3[:, QKV:]}
            RV4 = [p1[:, QP + QKV:QP + QKV + 4],
                   p1[:, QP + QKV + 4:QP + QKV + 8]]
            E4 = E4t[:, :4 * QP]   # E4[h,col] = 1 iff col in head h's block
            MTu = p2b[:, 0:QP]
            MTf = p2b[:, QP:QP + N]
            lnc_col = p2a[:, QKV:QKV + 1]
            degu_row = prow[0:1, 0:QP]
            degf_row = prow[0:1, QP:QP + N]
            o2 = QP + N
            bc_row = [prow[0:1, o2:o2 + C], prow[0:1, o2 + C:o2 + 2 * C]]



            def layer(L, x_in, sps, scat_rhs, scat_w):
                """x_in: SBUF [C, QP] bf16. Accumulates the layer output into
                the already-opened scatter PSUM tile `sps` ([:, :scat_w])."""
                G, VO = W[f"G{L}"], W[f"VO{L}"]

                # th = G_h^T x; head-pairs in two different banks so the
                # two PSUM->SBUF copies (DVE + ACT) can run in parallel
                # (the rv bias is applied key-side in the scores PSUM:
                # x^T(G^T x + rv 1^T) = x^T G^T x + (rv^T x)^T 1^T)
                # B[h,k] = rv_h . x_k first (its SBUF copy + the B-E4
                # scores opener are the longest pole before the scores MMs)
                Bp = psum.tile([128, 512], f32, tag="sc", bufs=1,
                               name=f"Bp{L}")
                nc.tensor.matmul(Bp[:4, :QP], RV4[L - 1], x_in,
                                 start=True, stop=True)
                Bsb = work.tile([4, QP], bf16, tag="Bsb")
                nc.vector.tensor_copy(out=Bsb[:], in_=Bp[:4, :QP])

                thp = [psum.tile([128, 256], f32, tag=t, bufs=1,
                                 name=f"thp{L}_{t}")
                       for t in ("th", "at")]
                for hp in range(2):
                    for hh in range(2):
                        h = hp * 2 + hh
                        nc.tensor.matmul(thp[hp][:, hh * QP:(hh + 1) * QP],
                                         G[:, h * C:(h + 1) * C], x_in,
                                         start=True, stop=True)

                if L == 1:
                    # hoist both scatter openers into the PE idle window
                    nc.tensor.matmul(sps[0][:, :QP], bc_row[0][:, :],
                                     degu_row[:, :], start=True, stop=False)
                    nc.tensor.matmul(sps[1][:, :N], bc_row[1][:, :],
                                     degf_row[:, :], start=True, stop=False)

                # scores group opener: B-bias via the E4 selector (before
                # vo so the scores matmuls are not queue-blocked behind it)
                sp = psum.tile([128, 2, 256], f32, tag="sc", bufs=1,
                               name=f"sp{L}")
                nc.tensor.matmul(
                    sp[:, :, :].rearrange("p a b -> p (a b)"),
                    Bsb[:], E4[:], start=True, stop=False)

                # vo = x^T VO (node-major), own bank
                vop = psum.tile([128, 512], f32, tag="vo", bufs=1,
                                name=f"vop{L}")
                nc.tensor.matmul(vop[:, :], x_in, VO[:], start=True, stop=True)

                # th PSUM -> SBUF: the two head-pair banks copy in
                # parallel on DVE and ACT into separate tiles
                th_sb = [work.tile([128, 2 * QP], bf16, tag=f"th_sb{hp}",
                                   name=f"th_sb{L}_{hp}") for hp in range(2)]
                nc.vector.tensor_copy(out=th_sb[0][:], in_=thp[0][:])
                nc.scalar.copy(out=th_sb[1][:], in_=thp[1][:])

                vt = work.tile([128, H, DH + 1], bf16, tag="vt")
                nc.vector.memset(vt[:, :, DH:], 1.0)
                nc.vector.tensor_copy(
                    out=vt[:, :, :DH],
                    in_=vop[:, :].rearrange("p (h c) -> p h c", h=H))

                # scores: close the accumulation group opened by the
                # B-bias opener with the two 256-col x^T th matmuls
                for hp in range(2):
                    nc.tensor.matmul(sp[:, hp, :], x_in,
                                     th_sb[hp][:],
                                     start=False, stop=(hp == 1))
                pt = work.tile([128, 2, 2 * QP], bf16, tag="PT")
                nc.scalar.activation(out=pt[:], in_=sp[:, :, :],
                                     func=AF.Exp, bias=lnc_col,
                                     scale=1.0)

                apt = psum.tile([128, 2, 2, 256], f32, tag="apx", bufs=1,
                                name=f"apt{L}")
                for b in range(2):
                    for hh in range(2):
                        nc.tensor.matmul(
                            apt[:, b, hh, :DH + 1],
                            pt[:, b, hh * QP:(hh + 1) * QP],
                            vt[:, 2 * b + hh, :], start=True, stop=True)

                # normalize (single reciprocal + broadcast multiply; one
                # DVE op each -- per-op dispatch overhead makes a b-split
                # slower), then the four scatter matmuls
                rz = work.tile([128, 2, 2, 1], f32, tag="rz")
                m = work.tile([128, 2, 2, DH], bf16, tag="msg")
                nc.vector.reciprocal(out=rz[:], in_=apt[:, :, :, DH:DH + 1])
                nc.vector.tensor_mul(m[:], apt[:, :, :, :DH],
                                     rz[:].broadcast_to([128, 2, 2, DH]))
                for b in range(2):
                    for hh in range(2):
                        nc.tensor.matmul(sps[L - 1][:, :scat_w],
                                         m[:, b, hh, :], scat_rhs[:, :scat_w],
                                         start=False,
                                         stop=(b == 1 and hh == 1))

            sps1 = psum.tile([128, 512], f32, tag="ot", bufs=2, name="sps1")
            sps2 = psum.tile([128, 512], f32, tag="ot", bufs=2, name="sps2")
            sps = [sps1, sps2]

            layer(1, xU, sps, MTu, QP)
            hU = work.tile([C, QP], bf16, tag="hU")
            nc.vector.tensor_scalar_max(out=hU[:], in0=sps1[:, :QP],
                                        scalar1=0.0)
            layer(2, hU, sps, MTf, N)

            # log_softmax over the node axis; |pre-softmax| is bounded so
            # exp is safe without max-subtraction.
            esum = work.tile([128, 1], f32, tag="esum")
            etmp = work.tile([128, N], bf16, tag="etmp")
            nc.scalar.activation(out=etmp[:], in_=sps2[:, :N], func=AF.Exp,
                                 scale=1.0, accum_out=esum[:])
            lse = work.tile([128, 1], f32, tag="lse")
            nc.scalar.activation(out=lse[:], in_=esum[:], func=AF.Ln)
            nc.vector.tensor_scalar_sub(out=out_sb[:], in0=sps2[:, :N],
                                        scalar1=lse[:])
            # data gate for the early-prepped writeback: the Pool copy
            # reads out_sb, so the trigger (queue-ordered after it) cannot
            # fire before the sub lands on real HW.
            gate = singles.tile([128, 1], f32, tag="w_gate")
            nc.gpsimd.tensor_copy(out=gate[:], in_=out_sb[:, 0:1])
            nc.gpsimd.trigger_dma(count=None)
            nc.gpsimd.wait_ge(ysem, 16)

    nc.compile()
    _patch_writeback_order(nc)
    _patch_dmasw(nc)
    return nc


def _prep_inputs_1c(x, edge_index, params, liveU):
    import ml_dtypes
    bf = ml_dtypes.bfloat16
    row = np.asarray(edge_index[0]).astype(np.int64)
    col = np.asarray(edge_index[1]).astype(np.int64)
    a = row[row]
    b = row[col]
    cb = np.bincount(b, minlength=N).astype(np.float64)
    M = np.zeros((N, N), np.float64)
    np.add.at(M, (col, a), 1.0)
    deg = np.bincount(col, minlength=N).astype(np.float64)

    folded = _fold_weights(params)

    xT = np.asarray(x, np.float64)[0].T
    nu = len(liveU)
    xU = np.zeros((C, QP)); xU[:, :nu] = xT[:, liveU]
    # exact integer counts (bf16-exact <= 256); dead/padded keys get a tiny
    # positive count so ln() stays finite and their softmax weight ~ 0
    cnt = np.full(QP, 2.0 ** -9)
    cbl = cb[liveU]
    cnt[:nu] = np.where(cbl > 0, cbl, 2.0 ** -9)
    MTu = np.zeros((C, QP)); MTu[:nu, :nu] = M[np.ix_(liveU, liveU)].T
    MTf = np.zeros((C, N)); MTf[:nu, :] = M[:, liveU].T
    degU = np.zeros(QP); degU[:nu] = deg[liveU]
    rv4 = [folded["RV1"].reshape(H, DH).T, folded["RV2"].reshape(H, DH).T]

    f8 = ml_dtypes.float8_e4m3fn
    pack1 = np.concatenate([xU, folded["G1"], rv4[0], rv4[1]], axis=1)
    packRow = np.concatenate(
        [degU, deg, folded["bc1"], folded["bc2"]])
    E4 = np.zeros((4, 4 * QP + packRow.shape[0]))
    for h in range(H):
        E4[h, h * QP:(h + 1) * QP] = 1.0
    E4[0, 4 * QP:] = packRow
    lnc = np.log(cnt.astype(ml_dtypes.bfloat16).astype(np.float64))
    pack2a = np.concatenate([folded["VO1"], lnc[:C, None]], axis=1)
    pack2b = np.concatenate([MTu, MTf], axis=1)
    pack3 = np.concatenate([folded["G2"], folded["VO2"]], axis=1)
    return {
        "pack1": np.ascontiguousarray(pack1.astype(f8)),
        "pack2a": np.ascontiguousarray(pack2a.astype(bf)),
        "packE4": np.ascontiguousarray(E4.astype(bf)),
        "pack2b": np.ascontiguousarray(pack2b.astype(bf)),
        "pack3": np.ascontiguousarray(pack3.astype(bf)),
    }


def _fold_weights(params):
    folded = {}
    scale = np.float64(1.0) / np.sqrt(np.float64(C))
    for L in (1, 2):
        p = {k: np.asarray(params[f"l{L}_{k}"]).astype(np.float64)
             for k in ("lin_w", "lin_b", "q_w", "q_b", "k_w", "k_b",
                       "v_w", "v_b", "o_w", "o_b")}
        sqlw = (p["q_w"] @ p["lin_w"]) * scale
        sqlb = (p["q_w"] @ p["lin_b"] + p["q_b"]) * scale
        klw = p["k_w"] @ p["lin_w"]
        vlw = p["v_w"] @ p["lin_w"]
        vlb = p["v_w"] @ p["lin_b"] + p["v_b"]
        G = np.empty((C, H * C))
        RV = np.empty(H * C)
        VO = np.empty((C, QKV))
        bo = np.empty(QKV)
        for h in range(H):
            sl = slice(h * DH, (h + 1) * DH)
            G[:, sl] = sqlw[sl].T @ klw[sl]
            RV[sl] = klw[sl].T @ sqlb[sl]
            ow_h = p["o_w"][:, sl]
            VO[:, sl] = vlw[sl].T @ ow_h.T
            bo[sl] = ow_h @ vlb[sl]
        folded[f"G{L}"] = G
        folded[f"RV{L}"] = RV
        folded[f"VO{L}"] = VO
        folded[f"bc{L}"] = bo.reshape(H, DH).sum(0) + p["o_b"]
    return folded


def kernel(x, edge_index, **params):
    from concourse.bass_utils import run_bass_kernel_spmd

    row = np.asarray(edge_index[0]).astype(np.int64)
    col = np.asarray(edge_index[1]).astype(np.int64)
    liveU = np.unique(np.concatenate([row[row], row[col]]))
    if len(liveU) <= QP:
        in_map = _prep_inputs_1c(x, edge_index, params, liveU)
        if "nc1c" not in _CACHE:
            _CACHE["nc1c"] = _build_program_1c()
        res = run_bass_kernel_spmd(_CACHE["nc1c"], [in_map] * N_CORES,
                                   core_ids=list(range(N_CORES)))
        yT = np.asarray(res.results[0]["yT"]).reshape(C, N)
        return np.ascontiguousarray(yT.T)[None].astype(np.float32)
    in_map = _prep_inputs(x, edge_index, params)
    res = run_on_device(in_map)
    yT = np.asarray(res.results[0]["yT"]).reshape(C, N)
    return np.ascontiguousarray(yT.T)[None].astype(np.float32)


def run_on_device(in_map, trace=False, **kwargs):
    from concourse.bass_utils import run_bass_kernel_spmd

    if "nc" not in _CACHE:
        _CACHE["nc"] = _build_program()
    nc = _CACHE["nc"]
    res = run_bass_kernel_spmd(nc, [in_map] * N_CORES,
                               core_ids=list(range(N_CORES)),
                               trace=trace, **kwargs)
    return res


def _build_program():
    """Multi-chunk fallback (union axis > 128): the original two-chunk
    program with separate query/key compactions on the full node axis."""
    import concourse.mybir as mybir
    import concourse.tile as tile
    from concourse import bacc

    f32 = mybir.dt.float32
    bf16 = mybir.dt.bfloat16
    AF = mybir.ActivationFunctionType

    nc = bacc.Bacc("TRN2", target_bir_lowering=False)

    din = {}
    for name, shape, dt_ in (
        ("packA", [C, PACKA], bf16),
        ("packB", [C, PACKB], bf16),
        ("packC", [C, PACKC], bf16),
        ("packRow", [1, PACKROW], bf16),
        ("lncF", [C, 2], f32),
    ):
        din[name] = nc.dram_tensor(name, shape, dt_, kind="ExternalInput")
    y_d = nc.dram_tensor("yT", [1, C, 1, N], f32, kind="ExternalOutput")

    with tile.TileContext(nc) as tc:
        with (
            tc.tile_pool(name="singles", bufs=1) as singles,
            tc.tile_pool(name="work", bufs=2) as work,
            tc.tile_pool(name="psum", bufs=1, space="PSUM") as psum,
        ):
            junk_bf = singles.tile([1, 512], bf16, tag="w_junk_bf")
            nc.gpsimd.memset(junk_bf[:, :288], 1.0)
            out_sb = singles.tile([128, N], f32, tag="w_out_sb")
            nc.vector.memset(out_sb[:], 0.0)
            zidx = singles.tile([128, 1], mybir.dt.int32, tag="w_zidx")
            nc.gpsimd.memset(zidx[:], 0)
            ysem = nc.alloc_semaphore("ysem")
            prow = singles.tile([1, PACKROW], bf16, tag="w_prow")
            nc.scalar.dma_start(prow[:], din["packRow"][:])
            ones_bf = singles.tile([1, N], bf16, tag="w_ones_bf")
            nc.gpsimd.memset(ones_bf[:], 1.0)
            for _ in range(9):
                ps = psum.tile([128, 512], f32, tag="b1", bufs=2)
                nc.tensor.matmul(ps[:, :288], junk_bf[:1, :128],
                                 junk_bf[:1, :288], start=True, stop=True)
            nc.scalar.add_instruction(mybir.InstLoadActFuncSet(
                act_func_set_id=6,
                name=nc.get_next_instruction_name(),
                ins=[], outs=[]))

            pA = singles.tile([C, PACKA], bf16, tag="w_pA")
            nc.sync.dma_start(pA[:], din["packA"][:])
            pB = singles.tile([C, PACKB], bf16, tag="w_pB")
            nc.sync.dma_start(pB[:, 0:QKV], din["packB"][:, 0:QKV])
            nc.sync.dma_start(pB[:, QKV:], din["packB"][:, QKV:])
            pC = singles.tile([C, PACKC], bf16, tag="w_pC")
            nc.sync.dma_start(pC[:], din["packC"][:])
            plnc = singles.tile([C, 2], f32, tag="w_plnc")
            nc.scalar.dma_start(plnc[:], din["lncF"][:])

            xT = pA[:, 0:N]
            W = {
                "G1": pA[:, N:N + QKV],
                "VO1": pB[:, 0:QKV],
                "G2": pC[:, 0:QKV],
                "VO2": pC[:, QKV:2 * QKV],
            }
            MT_sb = [pB[:, QKV:QKV + N], pB[:, QKV + N:QKV + 2 * N]]
            rv_row = [prow[0:1, 0:QKV], prow[0:1, QKV:2 * QKV]]
            deg_row = prow[0:1, 2 * QKV:2 * QKV + N]
            bc_row = [prow[0:1, 2 * QKV + N:2 * QKV + N + C],
                      prow[0:1, 2 * QKV + N + C:2 * QKV + N + 2 * C]]

            def gnn_layer(L, x_in):
                G = W[f"G{L}"]
                VO = W[f"VO{L}"]
                rvr = rv_row[L - 1]

                thp = [psum.tile([128, 512], f32, tag="b1", bufs=2,
                                 name=f"thp{hp}") for hp in range(2)]
                th_sb = work.tile([128, 2, 2 * N], bf16, tag="th_sb")
                for hp in range(2):
                    for hh in range(2):
                        h = hp * 2 + hh
                        nc.tensor.matmul(thp[hp][:, hh * N:(hh + 1) * N],
                                         G[:, h * C:(h + 1) * C], x_in,
                                         start=True, stop=False)
                        nc.tensor.matmul(thp[hp][:, hh * N:(hh + 1) * N],
                                         rvr[:, h * C:(h + 1) * C],
                                         ones_bf[:1, :N],
                                         start=False, stop=True)
                    eng = nc.vector.tensor_copy if hp == 0 else nc.scalar.copy
                    eng(out=th_sb[:, hp, :], in_=thp[hp][:, :2 * N])

                sp = []
                for ci, (w0, wc) in enumerate(_CHUNKS):
                    sp.append(psum.tile([128, 2, 512], f32, tag="b2", bufs=3,
                                        name=f"sp{ci}"))
                for hp in range(2):
                    for ci, (w0, wc) in enumerate(_CHUNKS):
                        nc.tensor.matmul(sp[ci][:wc, hp, :2 * N],
                                         x_in[:, w0:w0 + wc],
                                         th_sb[:, hp, :],
                                         start=True, stop=True)
                PT = []
                for ci, (w0, wc) in enumerate(_CHUNKS):
                    pt = work.tile([128, 2, 2 * N], bf16, tag=f"PT{ci}")
                    nc.scalar.activation(out=pt[:wc], in_=sp[ci][:wc, :, :2 * N],
                                         func=AF.Exp,
                                         bias=plnc[:wc, ci:ci + 1], scale=1.0)
                    PT.append(pt)

                vt = []
                for ci, (w0, wc) in enumerate(_CHUNKS):
                    ps = psum.tile([128, 512], f32, tag="b1", bufs=2)
                    nc.tensor.matmul(ps[:wc, :], x_in[:, w0:w0 + wc], VO[:],
                                     start=True, stop=True)
                    v = work.tile([128, H, DH + 1], bf16, tag=f"vt{ci}")
                    nc.vector.memset(v[:, :, DH:], 1.0)
                    nc.vector.tensor_copy(
                        out=v[:wc, :, :DH],
                        in_=ps[:wc, :].rearrange("p (h c) -> p h c", h=H))
                    vt.append(v)

                sps = psum.tile([128, 512], f32, tag="b1", bufs=2)
                nc.tensor.matmul(sps[:, :N], bc_row[L - 1][:, :],
                                 deg_row[:, :], start=True, stop=False)

                apt = [psum.tile([128, 2, 2, 256], f32, tag="b2", bufs=3,
                                 name=f"ap{ui}") for ui in range(2)]

                def att_mm(ui, b, hh, ci):
                    u0, uc = _CHUNKS[ui]
                    w0, wc = _CHUNKS[ci]
                    nc.tensor.matmul(
                        apt[ui][:uc, b, hh, :DH + 1],
                        PT[ci][:wc, b, hh * N + u0:hh * N + u0 + uc],
                        vt[ci][:wc, 2 * b + hh, :],
                        start=(ci == 0), stop=(ci == 1))

                for ui in range(2):
                    for b in range(2):
                        att_mm(ui, b, 0, 0)
                for ui in range(2):
                    for b in range(2):
                        att_mm(ui, b, 0, 1)
                        att_mm(ui, b, 1, 0)
                        att_mm(ui, b, 1, 1)
                ms, rzs = [], []
                for ui in range(2):
                    rzs.append(work.tile([128, 2, 2, 1], f32, tag=f"rz{ui}",
                                         name=f"rz{ui}"))
                    ms.append(work.tile([128, 2, 2, DH], bf16, tag=f"msg{ui}",
                                        name=f"msg{ui}"))
                nc.vector.reciprocal(out=rzs[0][:128],
                                     in_=apt[0][:128, :, :, DH:DH + 1])
                for b in range(2):
                    nc.vector.tensor_mul(
                        ms[0][:128, b], apt[0][:128, b, :, :DH],
                        rzs[0][:128, b].broadcast_to([128, 2, DH]))
                    for hh in range(2):
                        nc.tensor.matmul(sps[:, :N], ms[0][:128, b, hh, :],
                                         MT_sb[0][:128, :],
                                         start=False, stop=False)
                uc1 = _CHUNKS[1][1]
                nc.vector.reciprocal(out=rzs[1][:uc1],
                                     in_=apt[1][:uc1, :, :, DH:DH + 1])
                nc.vector.tensor_mul(
                    ms[1][:uc1], apt[1][:uc1, :, :, :DH],
                    rzs[1][:uc1].broadcast_to([uc1, 2, 2, DH]))
                for b in range(2):
                    for hh in range(2):
                        nc.tensor.matmul(sps[:, :N], ms[1][:uc1, b, hh, :],
                                         MT_sb[1][:uc1, :],
                                         start=False,
                                         stop=(b == 1 and hh == 1))
                return sps

            ps1 = gnn_layer(1, xT)
            hT = work.tile([C, N], bf16, tag="hT")
            nc.vector.tensor_scalar_max(out=hT[:], in0=ps1[:, :N], scalar1=0.0)
            ps2 = gnn_layer(2, hT)

            esum = work.tile([128, 1], f32, tag="esum")
            etmp = work.tile([128, N], bf16, tag="etmp")
            nc.scalar.activation(out=etmp[:], in_=ps2[:, :N], func=AF.Exp,
                                 scale=1.0, accum_out=esum[:])
            lse = work.tile([128, 1], f32, tag="lse")
            nc.scalar.activation(out=lse[:], in_=esum[:], func=AF.Ln)
            nc.vector.tensor_scalar_sub(out=out_sb[:], in0=ps2[:, :N],
                                        scalar1=lse[:])
            nc.gpsimd.kv_writeback(
                y_d[:], out_sb[:].rearrange("p (a b n) -> p a b n", a=1, b=1),
                zidx[:], prepare_only=True, sem=ysem)
            nc.gpsimd.trigger_dma(count=None)
            nc.gpsimd.wait_ge(ysem, 16)

    nc.compile()
    _patch_dmasw(nc)
    return nc


def _prep_inputs(x, edge_index, params):
    """Multi-chunk fallback host prep."""
    import ml_dtypes
    bf = ml_dtypes.bfloat16
    row = np.asarray(edge_index[0]).astype(np.int64)
    col = np.asarray(edge_index[1]).astype(np.int64)
    a = row[row]
    b = row[col]
    cb = np.bincount(b, minlength=N).astype(np.float64)
    lnc = np.where(cb > 0, np.log(np.maximum(cb, 1e-300)), -30000.0)
    M = np.zeros((N, N), np.float64)
    np.add.at(M, (col, a), 1.0)
    deg = np.bincount(col, minlength=N).astype(np.float64)

    folded = _fold_weights(params)

    xT = np.asarray(x, np.float64)[0].T
    MT = M.T
    MT0 = MT[0:128, :]
    MT1 = np.zeros((C, N))
    MT1[0:72, :] = MT[128:200, :]
    packA = np.concatenate([xT, folded["G1"]], axis=1)
    packB = np.concatenate([folded["VO1"], MT0, MT1], axis=1)
    packC = np.concatenate([folded["G2"], folded["VO2"]], axis=1)
    packRow = np.concatenate(
        [folded["RV1"], folded["RV2"], deg, folded["bc1"], folded["bc2"]]
    )[None, :]
    lncF = np.zeros((C, 2))
    lncF[0:128, 0] = lnc[0:128]
    lncF[0:72, 1] = lnc[128:200]
    assert packA.shape == (C, PACKA) and packB.shape == (C, PACKB)
    assert packC.shape == (C, PACKC) and packRow.shape == (1, PACKROW)
    return {
        "packA": np.ascontiguousarray(packA.astype(bf)),
        "packB": np.ascontiguousarray(packB.astype(bf)),
        "packC": np.ascontiguousarray(packC.astype(bf)),
        "packRow": np.ascontiguousarray(packRow.astype(bf)),
        "lncF": np.ascontiguousarray(lncF.astype(np.float32)),
    }


# revision 45
# speedup vs baseline: 1.0157x; 1.0157x over previous
"""Trainium2 Bass kernel for nn_AttentionalGNN (gnn_message_passing).

Algebraic collapse (exact): in the reference, src[e] = x[row[row[e]]] and
dst[e] = x[row[col[e]]], so the 4000x4000 edge attention collapses to a
200x200 node attention with multiplicative key weights cb[w] (applied as a
ln(cb) bias on the exp) and the scatter-add collapses to a 200x200 count
matrix M. lin/q/k/v/o fold on the host into G (Gram: scores are x^T G x),
RV (query-side bias), VO (o-projected values).

Fast path (_build_program_1c, 13916ns vs the 16328ns prior baseline):
liveQ = unique(row[row]) and liveK = unique(row[col]) are both subsets of
unique(row[:200]) (~126 of 200), so queries and keys share ONE <=128
union-compacted x: one x pack, one 128-wide L1 scatter + relu, and all
attention loops are single-chunk.

Timeline-model-driven layout (every stage verified at the TRN2 cost-model
floor -- DMA first-use pays desc-gen 625 + DGE delay 650 + sem-prop 900ns,
each cross-engine hop ~105-240ns):
* pack1 (x|G|rv4) is fp8-e4m3 (quantization adds ~1e-3 rel err, 14x under
  the gate; halves the first transfer). fp8 stationary x bf16 moving
  matmuls (scores, vo) are legal on HW. ln(count) ships precomputed as a
  bf16 bias column with VO1; counts stay exact.
* the rv bias never touches the th stage: scores = x^T(G^T x) + B^T 1
  with B = rv^T x (one 4-col matmul); B enters the scores PSUM as a
  rank-4 opener against a constant head-block selector E4, hidden in the
  PE idle window. E4 + all rank-1 rows ride one SWDGE transfer (Pool
  queue) in parallel with the serial HWDGE input stream.
* PSUM banks are all distinct (th-hp0 / th-hp1+apt-rotation / vo / sc
  (B,sp) / apt / sps1 / sps2 / warm-up), so the two th copies run
  genuinely parallel on DVE+ACT and nothing serializes on bank reuse.
* both scatter openers (bc deg rank-1s) hoist right after th; scatter
  accumulates into long-open groups closed by the 4 message matmuls.
* the kv_writeback descriptor prep (~1us SWDGE) runs at kernel start
  under the DMA shadow. Post-compile patches (the scheduler otherwise
  floats the dep-free trigger next to the prep, which would DMA stale
  zeros): the trigger gets the final sub's DVE-counter wait, the Pool
  gate copy is deleted (downstream Pool-counter waits decremented), and
  the ysem wait parks AFTER the final end-block barrier, so only the
  Pool queue rides out the 900ns DMA-completion propagation while every
  other queue retires early (kernel completion still implies the output
  DMA finished: Pool halts last). The framework's stale DMASW epilogue
  waits are stripped; ysem provides the ordering guarantee.
* ~11 warm-up matmuls on a spare bank bridge the PE p-state ramp on real
  HW (free in the cost model: the ramp anchors at PE queue start).
"""

import numpy as np

N = 200          # nodes
C = 128          # channels
H = 4            # heads
DH = 128         # head dim
QKV = 512        # H * DH
_CHUNKS = ((0, 128), (128, 72))   # multi-chunk fallback path
N_CORES = 8
QP = 128         # padded compact union axis (fast path)

PACKA = N + QKV             # mc: xT | G1
PACKB = QKV + 2 * N         # mc: VO1 | MT0 | MT1
PACKC = QKV + QKV           # mc: G2 | VO2
PACKROW = QKV + QKV + N + C + C   # mc: rv1 | rv2 | deg | bc1 | bc2

_CACHE = {}


def _patch_writeback_order(nc):
    """The scheduler orders the Pool queue as prep -> trigger -> wait(ysem)
    (the trigger has no data deps, so it floats right after the prep and
    would fire the output DMA with stale data). Give the trigger an explicit
    wait on the DVE engine counter at the final out_sb sub's completion
    value, drop the redundant Pool gate copy, and move trigger+wait to the
    end of the block."""
    import concourse.mybir as mybir

    for blk in nc.m.functions[0].blocks:
        insts = blk.instructions
        t_idx = next((i for i, x in enumerate(insts)
                      if type(x).__name__ == "InstTriggerDma"), None)
        if t_idx is None:
            continue
        trig = insts[t_idx]
        w_idx = next(i for i, x in enumerate(insts)
                     if isinstance(x, mybir.InstEventSemaphore)
                     and x.sync_info is not None
                     and any(w.ant_name == "ysem"
                             for w in (x.sync_info.on_wait or [])))
        wait = insts[w_idx]
        g_idx = next(i for i, x in enumerate(insts)
                     if isinstance(x, mybir.InstTensorCopy)
                     and getattr(x, "engine", None) == mybir.EngineType.Pool)
        gate = insts[g_idx]
        assert t_idx < w_idx < g_idx, (t_idx, w_idx, g_idx)
        # the gate copy's DVE wait IS the sub-completion condition; move it
        # onto the trigger itself and drop the gate
        dve_waits = [w for w in (gate.sync_info.on_wait or [])
                     if w.ant_name.startswith("DVE")]
        assert dve_waits, "gate copy lost its DVE wait"
        # ISA ops take a single sync wait: replace the Pool-counter wait
        # (prep ordering is already guaranteed by the queue: the prep
        # finishes ~10us before the sub lands) with the sub's DVE wait
        trig.sync_info = mybir.SyncInfo(
            on_wait=dve_waits,
            on_update=list(trig.sync_info.on_update or []))
        # the gate was a Pool_49 incrementer; find its ordinal among the
        # increments and decrement any wait thresholds that counted it
        ordinal = 0
        for b2 in nc.m.functions[0].blocks:
            done = False
            for x in b2.instructions:
                if x.sync_info is not None:
                    for u in (x.sync_info.on_update or []):
                        if u.ant_name == "Pool_49":
                            ordinal += u.update_value
                if x is gate:
                    done = True
                    break
            if done:
                break
        for b2 in nc.m.functions[0].blocks:
            for x in b2.instructions:
                if x is gate or x.sync_info is None:
                    continue
                for w in (x.sync_info.on_wait or []):
                    if w.ant_name == "Pool_49" and w.wait_value >= ordinal:
                        w.wait_value -= 1
        del insts[g_idx]
        del insts[w_idx]
        del insts[t_idx]
        # keep the block terminator (branch) last
        end = len(insts)
        while end > 0 and type(insts[end - 1]).__name__ in (
                "InstUnconditionalBranch", "InstEventSemaphore", "InstDrain"):
            end -= 1
        insts.insert(end, trig)
        # Park the ysem wait late in the END block (before the last Pool
        # barrier) so the ~900ns DMA-completion sem propagation overlaps
        # the epilogue drains instead of preceding them.
        blocks = nc.m.functions[0].blocks
        endblk = blocks[-1]
        # after the final barrier pair: only the Pool queue then rides
        # out the sem propagation; the kernel still cannot complete until
        # Pool has observed the DMA-completion semaphore
        endblk.instructions.append(wait)
        return
    raise AssertionError("trigger/gate/wait pattern not found")


def _patch_dmasw(nc):
    """Drop the framework's stale DMASW epilogue waits (the kv_writeback
    prep is tracked on the DMASW0 lane but completes on ysem; the explicit
    wait_ge(ysem) provides the ordering guarantee)."""
    import concourse.mybir as mybir

    for blk in nc.m.functions[0].blocks:
        for inst in blk.instructions:
            si = inst.sync_info
            if si is None or not isinstance(inst, mybir.InstEventSemaphore):
                continue
            waits = list(si.on_wait or [])
            keep = [w for w in waits
                    if not (w.ant_name or "").startswith("DMASW")]
            if len(keep) != len(waits):
                inst.sync_info = mybir.SyncInfo(
                    on_wait=keep, on_update=list(si.on_update or []))


def _build_program_1c():
    """Single-chunk union-compacted program (see module docstring)."""
    import concourse.mybir as mybir
    import concourse.tile as tile
    from concourse import bacc

    f32 = mybir.dt.float32
    bf16 = mybir.dt.bfloat16
    f8 = mybir.dt.float8e4
    AF = mybir.ActivationFunctionType

    P1 = QP + QKV + 8        # xU | G1 | RV4(L1) | RV4(L2)
    P2A = QKV + 1            # VO1 | ln(cnt)
    P2B = QP + N             # MTu | MTf
    P3 = 2 * QKV             # G2 | VO2
    PROW = QP + N + 2 * C    # degU | degF | bc1 | bc2 (row 0 of packE4row)
    PE4R = 4 * QP + PROW     # E4 | packRow-in-row-0

    nc = bacc.Bacc("TRN2", target_bir_lowering=False)

    din = {}
    for name, shape, dt_ in (
        ("pack1", [C, P1], f8),
        ("pack2a", [C, P2A], bf16),
        ("packE4", [4, PE4R], bf16),
        ("pack2b", [C, P2B], bf16),
        ("pack3", [C, P3], bf16),
    ):
        din[name] = nc.dram_tensor(name, shape, dt_, kind="ExternalInput")
    y_d = nc.dram_tensor("yT", [1, C, 1, N], f32, kind="ExternalOutput")

    with tile.TileContext(nc) as tc:
        with (
            tc.tile_pool(name="singles", bufs=1) as singles,
            tc.tile_pool(name="work", bufs=2) as work,
            tc.tile_pool(name="psum", bufs=1, space="PSUM") as psum,
        ):
            # --- Pool queue: packRow via SWDGE (parallel to HWDGE), then
            # the writeback descriptor prep under the DMA shadow ---
            E4t = singles.tile([4, PE4R], bf16, tag="w_E4")
            nc.gpsimd.dma_start(E4t[:], din["packE4"][:])
            prow = E4t[0:1, 4 * QP:]
            zidx = singles.tile([128, 1], mybir.dt.int32, tag="w_zidx")
            nc.gpsimd.memset(zidx[:], 0)
            ones_bf = singles.tile([1, N], bf16, tag="w_ones_bf")
            nc.vector.memset(ones_bf[:], 1.0)
            out_sb = singles.tile([128, N], f32, tag="w_out_sb")
            nc.vector.memset(out_sb[:], 0.0)
            ysem = nc.alloc_semaphore("ysem")
            nc.gpsimd.kv_writeback(
                y_d[:], out_sb[:].rearrange("p (a b n) -> p a b n", a=1, b=1),
                zidx[:], prepare_only=True, sem=ysem)

            # --- PE p-state ramp on a dedicated bank: one tile, closed
            # back-to-back groups (no pool rotation => no WAW semaphores),
            # keeps the PE busy from ~1us until the first weight DMA
            # lands (~3.35us) so the 3us clock ramp happens under the
            # DMA shadow ---
            jk = psum.tile([128, 512], f32, tag="apx", bufs=1)
            for _ in range(11):
                nc.tensor.matmul(jk[:, :N], ones_bf[:1, :128],
                                 ones_bf[:1, :N], start=True, stop=True)

            # ACT table: func-set 6 holds exp, ln AND copy.
            nc.scalar.add_instruction(mybir.InstLoadActFuncSet(
                act_func_set_id=6,
                name=nc.get_next_instruction_name(),
                ins=[], outs=[]))

            # --- HWDGE input DMAs, in need order (desc-gen serializes) ---
            p1 = singles.tile([C, P1], f8, tag="w_p1")
            nc.sync.dma_start(p1[:], din["pack1"][:])
            p2a = singles.tile([C, P2A], bf16, tag="w_p2a")
            nc.sync.dma_start(p2a[:], din["pack2a"][:])
            p2b = singles.tile([C, P2B], bf16, tag="w_p2b")
            nc.sync.dma_start(p2b[:], din["pack2b"][:])
            p3 = singles.tile([C, P3], bf16, tag="w_p3")
            nc.sync.dma_start(p3[:], din["pack3"][:])

            xU = p1[:, 0:QP]
            W = {"G1": p1[:, QP:QP + QKV], "VO1": p2a[:, 0:QKV],
                 "G2": p3[:, 0:QKV], "VO2": p3[:, QKV:]}
            RV4 = [p1[:, QP + QKV:QP + QKV + 4],
                   p1[:, QP + QKV + 4:QP + QKV + 8]]
            E4 = E4t[:, :4 * QP]   # E4[h,col] = 1 iff col in head h's block
            MTu = p2b[:, 0:QP]
            MTf = p2b[:, QP:QP + N]
            lnc_col = p2a[:, QKV:QKV + 1]
            degu_row = prow[0:1, 0:QP]
            degf_row = prow[0:1, QP:QP + N]
            o2 = QP + N
            bc_row = [prow[0:1, o2:o2 + C], prow[0:1, o2 + C:o2 + 2 * C]]



            def layer(L, x_in, sps, scat_rhs, scat_w):
                """x_in: SBUF [C, QP] bf16. Accumulates the layer output into
                the already-opened scatter PSUM tile `sps` ([:, :scat_w])."""
                G, VO = W[f"G{L}"], W[f"VO{L}"]

                # th = G_h^T x; head-pairs in two different banks so the
                # two PSUM->SBUF copies (DVE + ACT) can run in parallel
                # (the rv bias is applied key-side in the scores PSUM:
                # x^T(G^T x + rv 1^T) = x^T G^T x + (rv^T x)^T 1^T)
                # B[h,k] = rv_h . x_k first (its SBUF copy + the B-E4
                # scores opener are the longest pole before the scores MMs)
                Bp = psum.tile([128, 512], f32, tag="sc", bufs=1,
                               name=f"Bp{L}")
                nc.tensor.matmul(Bp[:4, :QP], RV4[L - 1], x_in,
                                 start=True, stop=True)
                Bsb = work.tile([4, QP], bf16, tag="Bsb")
                nc.vector.tensor_copy(out=Bsb[:], in_=Bp[:4, :QP])

                thp = [psum.tile([128, 256], f32, tag=t, bufs=1,
                                 name=f"thp{L}_{t}")
                       for t in ("th", "at")]
                for hp in range(2):
                    for hh in range(2):
                        h = hp * 2 + hh
                        nc.tensor.matmul(thp[hp][:, hh * QP:(hh + 1) * QP],
                                         G[:, h * C:(h + 1) * C], x_in,
                                         start=True, stop=True)

                if L == 1:
                    # hoist both scatter openers into the PE idle window
                    nc.tensor.matmul(sps[0][:, :QP], bc_row[0][:, :],
                                     degu_row[:, :], start=True, stop=False)
                    nc.tensor.matmul(sps[1][:, :N], bc_row[1][:, :],
                                     degf_row[:, :], start=True, stop=False)

                # scores group opener: B-bias via the E4 selector (before
                # vo so the scores matmuls are not queue-blocked behind it)
                sp = psum.tile([128, 2, 256], f32, tag="sc", bufs=1,
                               name=f"sp{L}")
                nc.tensor.matmul(
                    sp[:, :, :].rearrange("p a b -> p (a b)"),
                    Bsb[:], E4[:], start=True, stop=False)

                # vo = x^T VO (node-major), own bank
                vop = psum.tile([128, 512], f32, tag="vo", bufs=1,
                                name=f"vop{L}")
                nc.tensor.matmul(vop[:, :], x_in, VO[:], start=True, stop=True)

                # th PSUM -> SBUF: the two head-pair banks copy in
                # parallel on DVE and ACT into separate tiles
                th_sb = [work.tile([128, 2 * QP], bf16, tag=f"th_sb{hp}",
                                   name=f"th_sb{L}_{hp}") for hp in range(2)]
                nc.vector.tensor_copy(out=th_sb[0][:], in_=thp[0][:])
                nc.scalar.copy(out=th_sb[1][:], in_=thp[1][:])

                vt = work.tile([128, H, DH + 1], bf16, tag="vt")
                nc.vector.memset(vt[:, :, DH:], 1.0)
                nc.vector.tensor_copy(
                    out=vt[:, :, :DH],
                    in_=vop[:, :].rearrange("p (h c) -> p h c", h=H))

                # scores: close the accumulation group opened by the
                # B-bias opener with the two 256-col x^T th matmuls
                for hp in range(2):
                    nc.tensor.matmul(sp[:, hp, :], x_in,
                                     th_sb[hp][:],
                                     start=False, stop=(hp == 1))
                pt = work.tile([128, 2, 2 * QP], bf16, tag="PT")
                nc.scalar.activation(out=pt[:], in_=sp[:, :, :],
                                     func=AF.Exp, bias=lnc_col,
                                     scale=1.0)

                apt = psum.tile([128, 2, 2, 256], f32, tag="apx", bufs=1,
                                name=f"apt{L}")
                for b in range(2):
                    for hh in range(2):
                        nc.tensor.matmul(
                            apt[:, b, hh, :DH + 1],
                            pt[:, b, hh * QP:(hh + 1) * QP],
                            vt[:, 2 * b + hh, :], start=True, stop=True)

                # normalize (single reciprocal + broadcast multiply; one
                # DVE op each -- per-op dispatch overhead makes a b-split
                # slower), then the four scatter matmuls
                rz = work.tile([128, 2, 2, 1], f32, tag="rz")
                m = work.tile([128, 2, 2, DH], bf16, tag="msg")
                nc.vector.reciprocal(out=rz[:], in_=apt[:, :, :, DH:DH + 1])
                nc.vector.tensor_mul(m[:], apt[:, :, :, :DH],
                                     rz[:].broadcast_to([128, 2, 2, DH]))
                for b in range(2):
                    for hh in range(2):
                        nc.tensor.matmul(sps[L - 1][:, :scat_w],
                                         m[:, b, hh, :], scat_rhs[:, :scat_w],
                                         start=False,
                                         stop=(b == 1 and hh == 1))

            sps1 = psum.tile([128, 512], f32, tag="ot", bufs=2, name="sps1")
            sps2 = psum.tile([128, 512], f32, tag="ot", bufs=2, name="sps2")
            sps = [sps1, sps2]

            layer(1, xU, sps, MTu, QP)
            hU = work.tile([C, QP], bf16, tag="hU")
            nc.vector.tensor_scalar_max(out=hU[:], in0=sps1[:, :QP],
                                        scalar1=0.0)
            layer(2, hU, sps, MTf, N)

            # log_softmax over the node axis; |pre-softmax| is bounded so
            # exp is safe without max-subtraction.
            esum = work.tile([128, 1], f32, tag="esum")
            etmp = work.tile([128, N], bf16, tag="etmp")
            nc.scalar.activation(out=etmp[:], in_=sps2[:, :N], func=AF.Exp,
                                 scale=1.0, accum_out=esum[:])
            lse = work.tile([128, 1], f32, tag="lse")
            nc.scalar.activation(out=lse[:], in_=esum[:], func=AF.Ln)
            nc.vector.tensor_scalar_sub(out=out_sb[:], in0=sps2[:, :N],
                                        scalar1=lse[:])
            # data gate for the early-prepped writeback: the Pool copy
            # reads out_sb, so the trigger (queue-ordered after it) cannot
            # fire before the sub lands on real HW.
            gate = singles.tile([128, 1], f32, tag="w_gate")
            nc.gpsimd.tensor_copy(out=gate[:], in_=out_sb[:, 0:1])
            nc.gpsimd.trigger_dma(count=None)
            nc.gpsimd.wait_ge(ysem, 16)

    nc.compile()
    _patch_writeback_order(nc)
    _patch_dmasw(nc)
    return nc


def _prep_inputs_1c(x, edge_index, params, liveU):
    import ml_dtypes
    bf = ml_dtypes.bfloat16
    row = np.asarray(edge_index[0]).astype(np.int64)
    col = np.asarray(edge_index[1]).astype(np.int64)
    a = row[row]
    b = row[col]
    cb = np.bincount(b, minlength=N).astype(np.float64)
    M = np.zeros((N, N), np.float64)
    np.add.at(M, (col, a), 1.0)
    deg = np.bincount(col, minlength=N).astype(np.float64)

    folded = _fold_weights(params)

    xT = np.asarray(x, np.float64)[0].T
    nu = len(liveU)
    xU = np.zeros((C, QP)); xU[:, :nu] = xT[:, liveU]
    # exact integer counts (bf16-exact <= 256); dead/padded keys get a tiny
    # positive count so ln() stays finite and their softmax weight ~ 0
    cnt = np.full(QP, 2.0 ** -9)
    cbl = cb[liveU]
    cnt[:nu] = np.where(cbl > 0, cbl, 2.0 ** -9)
    MTu = np.zeros((C, QP)); MTu[:nu, :nu] = M[np.ix_(liveU, liveU)].T
    MTf = np.zeros((C, N)); MTf[:nu, :] = M[:, liveU].T
    degU = np.zeros(QP); degU[:nu] = deg[liveU]
    rv4 = [folded["RV1"].reshape(H, DH).T, folded["RV2"].reshape(H, DH).T]

    f8 = ml_dtypes.float8_e4m3fn
    pack1 = np.concatenate([xU, folded["G1"], rv4[0], rv4[1]], axis=1)
    packRow = np.concatenate(
        [degU, deg, folded["bc1"], folded["bc2"]])
    E4 = np.zeros((4, 4 * QP + packRow.shape[0]))
    for h in range(H):
        E4[h, h * QP:(h + 1) * QP] = 1.0
    E4[0, 4 * QP:] = packRow
    lnc = np.log(cnt.astype(ml_dtypes.bfloat16).astype(np.float64))
    pack2a = np.concatenate([folded["VO1"], lnc[:C, None]], axis=1)
    pack2b = np.concatenate([MTu, MTf], axis=1)
    pack3 = np.concatenate([folded["G2"], folded["VO2"]], axis=1)
    return {
        "pack1": np.ascontiguousarray(pack1.astype(f8)),
        "pack2a": np.ascontiguousarray(pack2a.astype(bf)),
        "packE4": np.ascontiguousarray(E4.astype(bf)),
        "pack2b": np.ascontiguousarray(pack2b.astype(bf)),
        "pack3": np.ascontiguousarray(pack3.astype(bf)),
    }


def _fold_weights(params):
    folded = {}
    scale = np.float64(1.0) / np.sqrt(np.float64(C))
    for L in (1, 2):
        p = {k: np.asarray(params[f"l{L}_{k}"]).astype(np.float64)
             for k in ("lin_w", "lin_b", "q_w", "q_b", "k_w", "k_b",
                       "v_w", "v_b", "o_w", "o_b")}
        sqlw = (p["q_w"] @ p["lin_w"]) * scale
        sqlb = (p["q_w"] @ p["lin_b"] + p["q_b"]) * scale
        klw = p["k_w"] @ p["lin_w"]
        vlw = p["v_w"] @ p["lin_w"]
        vlb = p["v_w"] @ p["lin_b"] + p["v_b"]
        G = np.empty((C, H * C))
        RV = np.empty(H * C)
        VO = np.empty((C, QKV))
        bo = np.empty(QKV)
        for h in range(H):
            sl = slice(h * DH, (h + 1) * DH)
            G[:, sl] = sqlw[sl].T @ klw[sl]
            RV[sl] = klw[sl].T @ sqlb[sl]
            ow_h = p["o_w"][:, sl]
            VO[:, sl] = vlw[sl].T @ ow_h.T
            bo[sl] = ow_h @ vlb[sl]
        folded[f"G{L}"] = G
        folded[f"RV{L}"] = RV
        folded[f"VO{L}"] = VO
        folded[f"bc{L}"] = bo.reshape(H, DH).sum(0) + p["o_b"]
    return folded


def kernel(x, edge_index, **params):
    from concourse.bass_utils import run_bass_kernel_spmd

    row = np.asarray(edge_index[0]).astype(np.int64)
    col = np.asarray(edge_index[1]).astype(np.int64)
    liveU = np.unique(np.concatenate([row[row], row[col]]))
    if len(liveU) <= QP:
        in_map = _prep_inputs_1c(x, edge_index, params, liveU)
        if "nc1c" not in _CACHE:
            _CACHE["nc1c"] = _build_program_1c()
        res = run_bass_kernel_spmd(_CACHE["nc1c"], [in_map] * N_CORES,
                                   core_ids=list(range(N_CORES)))
        yT = np.asarray(res.results[0]["yT"]).reshape(C, N)
        return np.ascontiguousarray(yT.T)[None].astype(np.float32)
    in_map = _prep_inputs(x, edge_index, params)
    res = run_on_device(in_map)
    yT = np.asarray(res.results[0]["yT"]).reshape(C, N)
    return np.ascontiguousarray(yT.T)[None].astype(np.float32)


def run_on_device(in_map, trace=False, **kwargs):
    from concourse.bass_utils import run_bass_kernel_spmd

    if "nc" not in _CACHE:
        _CACHE["nc"] = _build_program()
    nc = _CACHE["nc"]
    res = run_bass_kernel_spmd(nc, [in_map] * N_CORES,
                               core_ids=list(range(N_CORES)),
                               trace=trace, **kwargs)
    return res


def _build_program():
    """Multi-chunk fallback (union axis > 128): the original two-chunk
    program with separate query/key compactions on the full node axis."""
    import concourse.mybir as mybir
    import concourse.tile as tile
    from concourse import bacc

    f32 = mybir.dt.float32
    bf16 = mybir.dt.bfloat16
    AF = mybir.ActivationFunctionType

    nc = bacc.Bacc("TRN2", target_bir_lowering=False)

    din = {}
    for name, shape, dt_ in (
        ("packA", [C, PACKA], bf16),
        ("packB", [C, PACKB], bf16),
        ("packC", [C, PACKC], bf16),
        ("packRow", [1, PACKROW], bf16),
        ("lncF", [C, 2], f32),
    ):
        din[name] = nc.dram_tensor(name, shape, dt_, kind="ExternalInput")
    y_d = nc.dram_tensor("yT", [1, C, 1, N], f32, kind="ExternalOutput")

    with tile.TileContext(nc) as tc:
        with (
            tc.tile_pool(name="singles", bufs=1) as singles,
            tc.tile_pool(name="work", bufs=2) as work,
            tc.tile_pool(name="psum", bufs=1, space="PSUM") as psum,
        ):
            junk_bf = singles.tile([1, 512], bf16, tag="w_junk_bf")
            nc.gpsimd.memset(junk_bf[:, :288], 1.0)
            out_sb = singles.tile([128, N], f32, tag="w_out_sb")
            nc.vector.memset(out_sb[:], 0.0)
            zidx = singles.tile([128, 1], mybir.dt.int32, tag="w_zidx")
            nc.gpsimd.memset(zidx[:], 0)
            ysem = nc.alloc_semaphore("ysem")
            prow = singles.tile([1, PACKROW], bf16, tag="w_prow")
            nc.scalar.dma_start(prow[:], din["packRow"][:])
            ones_bf = singles.tile([1, N], bf16, tag="w_ones_bf")
            nc.gpsimd.memset(ones_bf[:], 1.0)
            for _ in range(9):
                ps = psum.tile([128, 512], f32, tag="b1", bufs=2)
                nc.tensor.matmul(ps[:, :288], junk_bf[:1, :128],
                                 junk_bf[:1, :288], start=True, stop=True)
            nc.scalar.add_instruction(mybir.InstLoadActFuncSet(
                act_func_set_id=6,
                name=nc.get_next_instruction_name(),
                ins=[], outs=[]))

            pA = singles.tile([C, PACKA], bf16, tag="w_pA")
            nc.sync.dma_start(pA[:], din["packA"][:])
            pB = singles.tile([C, PACKB], bf16, tag="w_pB")
            nc.sync.dma_start(pB[:, 0:QKV], din["packB"][:, 0:QKV])
            nc.sync.dma_start(pB[:, QKV:], din["packB"][:, QKV:])
            pC = singles.tile([C, PACKC], bf16, tag="w_pC")
            nc.sync.dma_start(pC[:], din["packC"][:])
            plnc = singles.tile([C, 2], f32, tag="w_plnc")
            nc.scalar.dma_start(plnc[:], din["lncF"][:])

            xT = pA[:, 0:N]
            W = {
                "G1": pA[:, N:N + QKV],
                "VO1": pB[:, 0:QKV],
                "G2": pC[:, 0:QKV],
                "VO2": pC[:, QKV:2 * QKV],
            }
            MT_sb = [pB[:, QKV:QKV + N], pB[:, QKV + N:QKV + 2 * N]]
            rv_row = [prow[0:1, 0:QKV], prow[0:1, QKV:2 * QKV]]
            deg_row = prow[0:1, 2 * QKV:2 * QKV + N]
            bc_row = [prow[0:1, 2 * QKV + N:2 * QKV + N + C],
                      prow[0:1, 2 * QKV + N + C:2 * QKV + N + 2 * C]]

            def gnn_layer(L, x_in):
                G = W[f"G{L}"]
                VO = W[f"VO{L}"]
                rvr = rv_row[L - 1]

                thp = [psum.tile([128, 512], f32, tag="b1", bufs=2,
                                 name=f"thp{hp}") for hp in range(2)]
                th_sb = work.tile([128, 2, 2 * N], bf16, tag="th_sb")
                for hp in range(2):
                    for hh in range(2):
                        h = hp * 2 + hh
                        nc.tensor.matmul(thp[hp][:, hh * N:(hh + 1) * N],
                                         G[:, h * C:(h + 1) * C], x_in,
                                         start=True, stop=False)
                        nc.tensor.matmul(thp[hp][:, hh * N:(hh + 1) * N],
                                         rvr[:, h * C:(h + 1) * C],
                                         ones_bf[:1, :N],
                                         start=False, stop=True)
                    eng = nc.vector.tensor_copy if hp == 0 else nc.scalar.copy
                    eng(out=th_sb[:, hp, :], in_=thp[hp][:, :2 * N])

                sp = []
                for ci, (w0, wc) in enumerate(_CHUNKS):
                    sp.append(psum.tile([128, 2, 512], f32, tag="b2", bufs=3,
                                        name=f"sp{ci}"))
                for hp in range(2):
                    for ci, (w0, wc) in enumerate(_CHUNKS):
                        nc.tensor.matmul(sp[ci][:wc, hp, :2 * N],
                                         x_in[:, w0:w0 + wc],
                                         th_sb[:, hp, :],
                                         start=True, stop=True)
                PT = []
                for ci, (w0, wc) in enumerate(_CHUNKS):
                    pt = work.tile([128, 2, 2 * N], bf16, tag=f"PT{ci}")
                    nc.scalar.activation(out=pt[:wc], in_=sp[ci][:wc, :, :2 * N],
                                         func=AF.Exp,
                                         bias=plnc[:wc, ci:ci + 1], scale=1.0)
                    PT.append(pt)

                vt = []
                for ci, (w0, wc) in enumerate(_CHUNKS):
                    ps = psum.tile([128, 512], f32, tag="b1", bufs=2)
                    nc.tensor.matmul(ps[:wc, :], x_in[:, w0:w0 + wc], VO[:],
                                     start=True, stop=True)
                    v = work.tile([128, H, DH + 1], bf16, tag=f"vt{ci}")
                    nc.vector.memset(v[:, :, DH:], 1.0)
                    nc.vector.tensor_copy(
                        out=v[:wc, :, :DH],
                        in_=ps[:wc, :].rearrange("p (h c) -> p h c", h=H))
                    vt.append(v)

                sps = psum.tile([128, 512], f32, tag="b1", bufs=2)
                nc.tensor.matmul(sps[:, :N], bc_row[L - 1][:, :],
                                 deg_row[:, :], start=True, stop=False)

                apt = [psum.tile([128, 2, 2, 256], f32, tag="b2", bufs=3,
                                 name=f"ap{ui}") for ui in range(2)]

                def att_mm(ui, b, hh, ci):
                    u0, uc = _CHUNKS[ui]
                    w0, wc = _CHUNKS[ci]
                    nc.tensor.matmul(
                        apt[ui][:uc, b, hh, :DH + 1],
                        PT[ci][:wc, b, hh * N + u0:hh * N + u0 + uc],
                        vt[ci][:wc, 2 * b + hh, :],
                        start=(ci == 0), stop=(ci == 1))

                for ui in range(2):
                    for b in range(2):
                        att_mm(ui, b, 0, 0)
                for ui in range(2):
                    for b in range(2):
                        att_mm(ui, b, 0, 1)
                        att_mm(ui, b, 1, 0)
                        att_mm(ui, b, 1, 1)
                ms, rzs = [], []
                for ui in range(2):
                    rzs.append(work.tile([128, 2, 2, 1], f32, tag=f"rz{ui}",
                                         name=f"rz{ui}"))
                    ms.append(work.tile([128, 2, 2, DH], bf16, tag=f"msg{ui}",
                                        name=f"msg{ui}"))
                nc.vector.reciprocal(out=rzs[0][:128],
                                     in_=apt[0][:128, :, :, DH:DH + 1])
                for b in range(2):
                    nc.vector.tensor_mul(
                        ms[0][:128, b], apt[0][:128, b, :, :DH],
                        rzs[0][:128, b].broadcast_to([128, 2, DH]))
                    for hh in range(2):
                        nc.tensor.matmul(sps[:, :N], ms[0][:128, b, hh, :],
                                         MT_sb[0][:128, :],
                                         start=False, stop=False)
                uc1 = _CHUNKS[1][1]
                nc.vector.reciprocal(out=rzs[1][:uc1],
                                     in_=apt[1][:uc1, :, :, DH:DH + 1])
                nc.vector.tensor_mul(
                    ms[1][:uc1], apt[1][:uc1, :, :, :DH],
                    rzs[1][:uc1].broadcast_to([uc1, 2, 2, DH]))
                for b in range(2):
                    for hh in range(2):
                        nc.tensor.matmul(sps[:, :N], ms[1][:uc1, b, hh, :],
                                         MT_sb[1][:uc1, :],
                                         start=False,
                                         stop=(b == 1 and hh == 1))
                return sps

            ps1 = gnn_layer(1, xT)
            hT = work.tile([C, N], bf16, tag="hT")
            nc.vector.tensor_scalar_max(out=hT[:], in0=ps1[:, :N], scalar1=0.0)
            ps2 = gnn_layer(2, hT)

            esum = work.tile([128, 1], f32, tag="esum")
            etmp = work.tile([128, N], bf16, tag="etmp")
            nc.scalar.activation(out=etmp[:], in_=ps2[:, :N], func=AF.Exp,
                                 scale=1.0, accum_out=esum[:])
            lse = work.tile([128, 1], f32, tag="lse")
            nc.scalar.activation(out=lse[:], in_=esum[:], func=AF.Ln)
            nc.vector.tensor_scalar_sub(out=out_sb[:], in0=ps2[:, :N],
                                        scalar1=lse[:])
            nc.gpsimd.kv_writeback(
                y_d[:], out_sb[:].rearrange("p (a b n) -> p a b n", a=1, b=1),
                zidx[:], prepare_only=True, sem=ysem)
            nc.gpsimd.trigger_dma(count=None)
            nc.gpsimd.wait_ge(ysem, 16)

    nc.compile()
    _patch_dmasw(nc)
    return nc


def _prep_inputs(x, edge_index, params):
    """Multi-chunk fallback host prep."""
    import ml_dtypes
    bf = ml_dtypes.bfloat16
    row = np.asarray(edge_index[0]).astype(np.int64)
    col = np.asarray(edge_index[1]).astype(np.int64)
    a = row[row]
    b = row[col]
    cb = np.bincount(b, minlength=N).astype(np.float64)
    lnc = np.where(cb > 0, np.log(np.maximum(cb, 1e-300)), -30000.0)
    M = np.zeros((N, N), np.float64)
    np.add.at(M, (col, a), 1.0)
    deg = np.bincount(col, minlength=N).astype(np.float64)

    folded = _fold_weights(params)

    xT = np.asarray(x, np.float64)[0].T
    MT = M.T
    MT0 = MT[0:128, :]
    MT1 = np.zeros((C, N))
    MT1[0:72, :] = MT[128:200, :]
    packA = np.concatenate([xT, folded["G1"]], axis=1)
    packB = np.concatenate([folded["VO1"], MT0, MT1], axis=1)
    packC = np.concatenate([folded["G2"], folded["VO2"]], axis=1)
    packRow = np.concatenate(
        [folded["RV1"], folded["RV2"], deg, folded["bc1"], folded["bc2"]]
    )[None, :]
    lncF = np.zeros((C, 2))
    lncF[0:128, 0] = lnc[0:128]
    lncF[0:72, 1] = lnc[128:200]
    assert packA.shape == (C, PACKA) and packB.shape == (C, PACKB)
    assert packC.shape == (C, PACKC) and packRow.shape == (1, PACKROW)
    return {
        "packA": np.ascontiguousarray(packA.astype(bf)),
        "packB": np.ascontiguousarray(packB.astype(bf)),
        "packC": np.ascontiguousarray(packC.astype(bf)),
        "packRow": np.ascontiguousarray(packRow.astype(bf)),
        "lncF": np.ascontiguousarray(lncF.astype(np.float32)),
    }
